# revision 53
# baseline (speedup 1.0000x reference)
"""EquiMHA Trainium2 kernel.

Data-parallel over batch B=8 across the 8 NeuronCores (one batch element per
core, weights replicated, no collectives).

Per-core computation for batch b (N=512, M=4, KN=512, DQ=DK=512, DV=1024,
H=16, D=64):
  Qp = Q[b] @ w_q, Kp = K[b] @ w_k, Vp = K[b] @ w_v
  E[h,n,k] = sum_{m,d} Qp[n,m,h*64+d] Kp[k,m,h*64+d] / 32
  A = masked_softmax(E)        (max-subtraction skipped: |E|/32 <= ~2, and the
                                max cancels exactly up to the +eps term)
  O[n,m,h*64+d] = sum_k A[h,n,k] Vp[k,m,h*64+d]
  out = O @ w_o

Precision strategy (tolerance is 2e-2; measured pipeline error ~8e-3):
  - Q/K-side projections run in fp8e4m3 with DoubleRow perf mode (2x PE
    rate, 256-deep contraction per pass). Host pre-quantizes Q, K and
    w_q, w_k (weights pre-scaled by 64 into fp8 normal range; the 64*64
    factor is folded into the exp scale).
  - Everything downstream (E scores, A@V, O@w_o) runs in bf16, which is
    full PE rate and halves SBUF/DMA vs f32 so the whole working set
    stays on-chip (no DRAM round trips).

Layout strategy: the host pre-transposes all inputs into the exact tile
layouts the PE wants, so the kernel does zero on-chip transposes:
  - QT8/KT8 [256,2,M,N]: [c*128+p, i, m, n] with dq = c*256+i*128+p, the
    DoubleRow pairing of two 128-deep contraction chunks.
  - P1/P2 emit qpp/kpp[h] = [(s,d) 128, mp, n|k] bf16 via partition-shifted
    psum evacuation (DoubleRow outputs land on psum partitions 0-63).
  - E^T[k,n] per head is a 2-matmul bf16 contraction over mp; softmax runs
    in [k, n] orientation entirely OFF the PE: exp on ACT (bf16 out), mask
    multiply + a bf16 partial-sum tree on DVE (2-byte fast mode), a Pool
    partition_all_reduce for the column sums (output already broadcast
    across partitions), and a bf16 DVE reciprocal.
  - Vp2[kc] = [128 k, (h, mp, s, d) 4096] bf16 so the O matmul stationary
    is one contiguous 128-column slice; O psum is normalized on DVE
    directly into OT[hp] = [(h%2,d), m, n] bf16 tiles, which are exactly
    the stationary operands P7 wants. Output leaves in natural [n, m, dvo]
    orientation for contiguous stores.
"""

import numpy as np
import ml_dtypes

import concourse.bacc as bacc
import concourse.mybir as mybir
import concourse.tile as tile

F32 = mybir.dt.float32
F32R = mybir.dt.float32r
F8 = mybir.dt.float8e4
BF = mybir.dt.bfloat16
AF = mybir.ActivationFunctionType
DR = mybir.MatmulPerfMode.DoubleRow

NPF8 = ml_dtypes.float8_e4m3
NPBF = ml_dtypes.bfloat16

B, N, M, KN = 8, 512, 4, 512
DQ, DK, DV, H = 512, 512, 1024, 16
D = DV // H
HP = H // 2          # head pairs (P7 contraction chunks)
KC = KN // 128       # k chunks
NC = N // 128        # n chunks
WS = 64.0            # host pre-scale for fp8 weights
SCALE = 1.0 / 32.0   # 1/sqrt(DV)
ESCALE = SCALE / (WS * WS)  # fused into exp


def build_nc():
    nc = bacc.Bacc("TRN2", target_bir_lowering=False, debug=False, num_devices=8)

    qt8_d = nc.dram_tensor("QT8", [256, 2, M, N], F8, kind="ExternalInput")
    kt8_d = nc.dram_tensor("KT8", [256, 2, M, KN], F8, kind="ExternalInput")
    ktb_d = nc.dram_tensor("KTB", [DK, M, KN], BF, kind="ExternalInput")
    mt_d = nc.dram_tensor("MT", [KN, N], BF, kind="ExternalInput")
    wq8_d = nc.dram_tensor("WQ8", [256, 2, DV], F8, kind="ExternalInput")
    wk8_d = nc.dram_tensor("WK8", [256, 2, DV], F8, kind="ExternalInput")
    wvb_d = nc.dram_tensor("WVB", [DK, DV], BF, kind="ExternalInput")
    wob_d = nc.dram_tensor("WOB", [DV, DV], BF, kind="ExternalInput")
    out_d = nc.dram_tensor("out", [N, M, DV], F32, kind="ExternalOutput")

    def ecopy(eng, dst, src):
        if eng is nc.scalar:
            nc.scalar.copy(dst, src)
        else:
            eng.tensor_copy(dst, src)

    with tile.TileContext(nc) as tc:
        with tc.tile_pool(name="persist", bufs=1) as persist:
            maskT2 = [persist.tile([128, 2, N], BF, name=f"mT{p}") for p in range(2)]
            vp2 = [persist.tile([128, M * DV], BF, name=f"vp2_{kc}") for kc in range(KC)]
            qpp = [persist.tile([128, 2, N], BF, name=f"qpp{h}") for h in range(H)]
            kpp = [persist.tile([128, 2, KN], BF, name=f"kpp{h}") for h in range(H)]
            ot = [persist.tile([128, M, N], BF, name=f"ot{hp}") for hp in range(HP)]
            wob = [persist.tile([128, DV], BF, name=f"wob{hp}") for hp in range(HP)]

            # ============ projections ============
            # Phase A: P2 (fp8 DR, evac-heavy) interleaved with half of P3
            # (bf16, PE-heavy); phase B: P1 interleaved with the other half.
            # Interleaving keeps the psum-evac engines (ACT/DVE) loaded
            # continuously instead of bursting past PE per phase.
            eng_ns = {"act": 0.0, "dve": 0.0}

            def pick_eng(act_cost, dve_cost):
                if eng_ns["act"] + act_cost <= eng_ns["dve"] + dve_cost:
                    eng_ns["act"] += act_cost
                    return nc.scalar
                eng_ns["dve"] += dve_cost
                return nc.vector

            with (
                tc.tile_pool(name="xk8", bufs=1) as xk8p,
                tc.tile_pool(name="w8", bufs=1) as w8p,
                tc.tile_pool(name="xkb", bufs=1) as xkbp,
                tc.tile_pool(name="wvb", bufs=1) as wvbp,
                tc.tile_pool(name="ppj", bufs=3, space="PSUM") as ppj,
                tc.tile_pool(name="ppv", bufs=2, space="PSUM") as ppv,
            ):
                xk8 = [xk8p.tile([128, 2, M, KN], F8, name=f"xk8_{c}") for c in range(2)]
                wk8 = [w8p.tile([128, 2, DV], F8, name=f"wk8_{c}") for c in range(2)]
                xkb = [xkbp.tile([128, M, KN], BF, name=f"xkb{c}") for c in range(4)]
                wvb = [wvbp.tile([128, DV], BF, name=f"wvb{c}") for c in range(4)]
                xq8 = [xk8p.tile([128, 2, M, N], F8, name=f"xq8_{c}") for c in range(2)]
                wq8 = [w8p.tile([128, 2, DV], F8, name=f"wq8_{c}") for c in range(2)]
                # interleaved so the first matmul's operands land first:
                # wk8[0], m0c0, wk8[1], m0c1, then the rest of the s=0 set
                nc.sync.dma_start(wk8[0], wk8_d.ap()[0:128])
                nc.sync.dma_start(xk8[0][:, :, 0, :], kt8_d.ap()[0:128, :, 0, :])
                nc.sync.dma_start(wk8[1], wk8_d.ap()[128:256])
                nc.sync.dma_start(xk8[1][:, :, 0, :], kt8_d.ap()[128:256, :, 0, :])
                for m in (2, 1, 3):
                    for c in range(2):
                        nc.sync.dma_start(
                            xk8[c][:, :, m, :],
                            kt8_d.ap()[c * 128 : (c + 1) * 128, :, m, :],
                        )
                for c in range(4):
                    nc.sync.dma_start(wvb[c], wvb_d.ap()[c * 128 : (c + 1) * 128])
                    nc.sync.dma_start(xkb[c], ktb_d.ap()[c * 128 : (c + 1) * 128])
                for c in range(2):
                    nc.sync.dma_start(wq8[c], wq8_d.ap()[c * 128 : (c + 1) * 128])
                    nc.sync.dma_start(xq8[c], qt8_d.ap()[c * 128 : (c + 1) * 128])
                for kc in range(KC):
                    nc.sync.dma_start(
                        maskT2[kc // 2][:, kc % 2, :],
                        mt_d.ap()[kc * 128 : (kc + 1) * 128],
                    )
                for hp in range(HP):
                    nc.sync.dma_start(wob[hp], wob_d.ap()[hp * 128 : (hp + 1) * 128])

                def proj8(h, s, w_sb, x_sb, dst):
                    pq2 = ppj.tile([64, 1024], F32, name="pq2", tag="pq2")
                    for mp in range(2):
                        m = 2 * mp + s
                        for nh in range(2):
                            for c in range(2):
                                nc.tensor.matmul(
                                    pq2[:, mp * 512 + nh * 256 : mp * 512 + (nh + 1) * 256],
                                    w_sb[c][:, :, h * 64 : (h + 1) * 64],
                                    x_sb[c][:, :, m, nh * 256 : (nh + 1) * 256],
                                    start=(c == 0),
                                    stop=(c == 1),
                                    perf_mode=DR,
                                )
                    ecopy(
                        pick_eng(1030, 1310),
                        dst[s * 64 : (s + 1) * 64, :, :],
                        pq2.rearrange("p (t n) -> p t n", t=2),
                    )

                p3_units = [
                    (mi, kc, dvh)
                    for mi in range(M)
                    for kc in range(KC)
                    for dvh in range(2)
                ]

                def p3_unit(u):
                    mi, kc, dvh = u
                    off = (mi // 2) * 128 + (mi % 2) * 64
                    pv = ppv.tile([128, 512], F32, name="pv", tag="pv")
                    for c in range(4):
                        nc.tensor.matmul(
                            pv,
                            xkb[c][:, mi, kc * 128 : (kc + 1) * 128],
                            wvb[c][:, dvh * 512 : (dvh + 1) * 512],
                            start=(c == 0),
                            stop=(c == 3),
                        )
                    v4 = vp2[kc].rearrange("p (h c) -> p h c", h=H)
                    ecopy(
                        pick_eng(610, 730),
                        v4[:, dvh * 8 : (dvh + 1) * 8, off : off + 64],
                        pv.rearrange("p (h d) -> p h d", h=8),
                    )

                p3i = 0
                # phase A: P2, s-major (the s=0 pass needs only half of K);
                # two P3 units per head in the second pass
                for h in range(H):
                    proj8(h, 0, wk8, xk8, kpp[h])
                for h in range(H):
                    proj8(h, 1, wk8, xk8, kpp[h])
                    if h >= 8:
                        p3_unit(p3_units[p3i]); p3i += 1
                        p3_unit(p3_units[p3i]); p3i += 1
                # phase B: P1 x16 heads; one P3 unit per head
                for h in range(H):
                    proj8(h, 0, wq8, xq8, qpp[h])
                    proj8(h, 1, wq8, xq8, qpp[h])
                    if p3i < len(p3_units):
                        p3_unit(p3_units[p3i]); p3i += 1
                while p3i < len(p3_units):
                    p3_unit(p3_units[p3i]); p3i += 1

            # ============ attention, per head (software-pipelined) ============
            # Depth-3 pipeline: iteration h emits norm(h-3), sums(h-1)'s
            # add/all-reduce, E(h), recip(h-1), then O(h-2) — each engine's
            # queue only ever holds instructions whose operands are already
            # (or imminently) ready, so the cross-engine softmax chain
            # (exp -> mask-mul -> adds -> all-reduce -> recip -> normalize)
            # never stalls the PE or head-of-line-blocks DVE. Softmax sums
            # run OFF the PE: bf16 tree-adds on DVE (2-byte fast mode), a
            # Pool partition_all_reduce (output already broadcast across
            # partitions), and a bf16 DVE reciprocal.
            import concourse.bass_isa as bass_isa

            with (
                tc.tile_pool(name="exp", bufs=4) as expp,
                tc.tile_pool(name="emp", bufs=12) as emp,
                tc.tile_pool(name="sump", bufs=2) as sump,
                tc.tile_pool(name="repp", bufs=3) as repp,
                tc.tile_pool(name="opop", bufs=3) as opop,
                tc.tile_pool(name="pse", bufs=2, space="PSUM") as pse,
                tc.tile_pool(name="pso", bufs=2, space="PSUM") as pso,  # 2x2+2x2 = 8
            ):

                def emit_e(h):
                    # E psum lands in 2-bank [128, 1024] pairs so one ACT
                    # exp covers two k-chunks (halves ACT instruction count)
                    em = []
                    for p in range(2):
                        pe2 = pse.tile([128, 2 * N], F32, name="pe", tag="pe")
                        for half in range(2):
                            kc = 2 * p + half
                            for mp in range(2):
                                nc.tensor.matmul(
                                    pe2[:, half * N : (half + 1) * N],
                                    kpp[h][:, mp, kc * 128 : (kc + 1) * 128],
                                    qpp[h][:, mp, :],
                                    start=(mp == 0),
                                    stop=(mp == 1),
                                )
                        ex2 = expp.tile([128, 2, N], BF, name="ex", tag="ex")
                        nc.scalar.activation(
                            ex2.rearrange("p t n -> p (t n)"),
                            pe2,
                            AF.Exp,
                            scale=ESCALE,
                        )
                        for half in range(2):
                            emt = emp.tile([128, N], BF, name="em", tag="em")
                            nc.vector.tensor_mul(
                                emt, ex2[:, half, :], maskT2[p][:, half, :]
                            )
                            em.append(emt)
                    return em

                def emit_sums(h, em):
                    """bf16 tree-adds + Pool all-reduce (recip emitted later)"""
                    t0 = sump.tile([128, N], BF, name="t0", tag="t0")
                    nc.vector.tensor_add(t0, em[0], em[1])
                    t1 = sump.tile([128, N], BF, name="t1", tag="t1")
                    nc.vector.tensor_add(t1, em[2], em[3])
                    s_all = sump.tile([128, N], BF, name="s_all", tag="t0")
                    nc.vector.tensor_add(s_all, t0, t1)
                    s_red = sump.tile([128, N], BF, name="s_red", tag="t1")
                    nc.gpsimd.partition_all_reduce(
                        s_red, s_all, channels=128, reduce_op=bass_isa.ReduceOp.add
                    )
                    return s_red

                def emit_recip(s_red):
                    rep = repp.tile([128, N], BF, name="rep", tag="rep")
                    with nc.allow_low_precision(reason="softmax 1/sum"):
                        nc.vector.reciprocal(rep, s_red)
                    return rep

                def emit_o(h, em):
                    """O matmuls into a 2-bank psum pair; one ACT evac"""
                    po2 = pso.tile([128, 2 * N], F32, name="po", tag="po")
                    for mp in range(2):
                        for kc in range(KC):
                            nc.tensor.matmul(
                                po2[:, mp * N : (mp + 1) * N],
                                vp2[kc][:, h * 256 + mp * 128 : h * 256 + (mp + 1) * 128],
                                em[kc],
                                start=(kc == 0),
                                stop=(kc == KC - 1),
                            )
                    opo = opop.tile([128, 2, N], BF, name="opo", tag="opo")
                    nc.scalar.copy(
                        opo.rearrange("p t n -> p (t n)"), po2
                    )
                    return opo

                def emit_norm(h, opo, rep):
                    """normalize O into OT tiles (bf16 fast muls, DVE/Pool;
                    late heads all-DVE so the P7 warm-up isn't gated on Pool)"""
                    hp, hs = h // 2, h % 2
                    for mp in range(2):
                        for s in range(2):
                            eng = nc.gpsimd if (mp == 1 and h < 13) else nc.vector
                            eng.tensor_mul(
                                ot[hp][hs * 64 : (hs + 1) * 64, 2 * mp + s, :],
                                opo[s * 64 : (s + 1) * 64, mp, :],
                                rep[s * 64 : (s + 1) * 64, :],
                            )

                em_q, sr_q, rep_q, opo_q = {}, {}, {}, {}
                for h in range(H):
                    if h >= 3:
                        emit_norm(h - 3, opo_q.pop(h - 3), rep_q[h - 3])
                    if h >= 1:
                        sr_q[h - 1] = emit_sums(h - 1, em_q[h - 1])
                    if h >= 14:
                        # last iterations: O before E so the opo evacs reach
                        # ACT ahead of the exps and free the psum pairs the
                        # epilogue's O(14)/O(15) need
                        opo_q[h - 2] = emit_o(h - 2, em_q.pop(h - 2))
                    em_q[h] = emit_e(h)
                    if h >= 1:
                        rep_q[h - 1] = emit_recip(sr_q.pop(h - 1))
                    if 2 <= h < 14:
                        opo_q[h - 2] = emit_o(h - 2, em_q.pop(h - 2))
                # epilogue: h-1=15 sums/recip, O(14), O(15), norms 13..15
                sr_q[15] = emit_sums(15, em_q[15])
                rep_q[15] = emit_recip(sr_q.pop(15))
                opo_q[14] = emit_o(14, em_q.pop(14))
                emit_norm(13, opo_q.pop(13), rep_q[13])
                opo_q[15] = emit_o(15, em_q.pop(15))
                emit_norm(14, opo_q.pop(14), rep_q[14])
                emit_norm(15, opo_q.pop(15), rep_q[15])

            # ============ P7: output projection (bf16) ============
            # The first four psum groups run contraction chunks hp=0..5
            # before any hp>=6 step, so the PE keeps streaming while the
            # last two heads\' normalized OT tiles are still being written.
            with (
                tc.tile_pool(name="outst", bufs=4) as outstp,
                tc.tile_pool(name="psf", bufs=4, space="PSUM") as psf,
            ):
                units = [(mi, ncc) for mi in range(M) for ncc in range(NC)]

                def p7_mms(pf, mi, ncc, dvh, hps, first, last):
                    for hp in hps:
                        nc.tensor.matmul(
                            pf,
                            ot[hp][:, mi, ncc * 128 : (ncc + 1) * 128],
                            wob[hp][:, dvh * 512 : (dvh + 1) * 512],
                            start=(hp == first),
                            stop=(hp == last),
                        )

                def p7_finish(u, pf0, pf1):
                    mi, ncc = u
                    ost = outstp.tile([128, 2, 512], F32, name="ost", tag="ost")
                    nc.scalar.copy(ost[:, 0, :], pf0)
                    nc.vector.tensor_copy(ost[:, 1, :], pf1)
                    nc.sync.dma_start(
                        out_d.ap()[ncc * 128 : (ncc + 1) * 128, mi, :],
                        ost.rearrange("p a b -> p (a b)"),
                    )

                # warm stretch: 4 psum groups of hp0..5 for the first 2 units
                warm = []
                for u in units[:4]:
                    mi, ncc = u
                    pfs = []
                    for dvh in range(2):
                        pf = psf.tile([128, 512], F32, name="pf", tag="pf")
                        p7_mms(pf, mi, ncc, dvh, range(6), 0, HP - 1)
                        pfs.append(pf)
                    warm.append((u, pfs))
                for u, pfs in warm:
                    mi, ncc = u
                    for dvh in range(2):
                        p7_mms(pfs[dvh], mi, ncc, dvh, range(6, HP), 0, HP - 1)
                    p7_finish(u, *pfs)
                for u in units[4:-1]:
                    mi, ncc = u
                    pfs = []
                    for dvh in range(2):
                        pf = psf.tile([128, 512], F32, name="pf", tag="pf")
                        p7_mms(pf, mi, ncc, dvh, range(HP), 0, HP - 1)
                        pfs.append(pf)
                    p7_finish(u, *pfs)
                # last unit: quarter-column evac/DMA pipeline for a
                # short end-of-kernel drain
                mi, ncc = units[-1]
                for dvh in range(2):
                    pf = psf.tile([128, 512], F32, name="pf", tag="pf")
                    p7_mms(pf, mi, ncc, dvh, range(HP), 0, HP - 1)
                    osh = outstp.tile([128, 512], F32, name="osh", tag="osh")
                    eng = nc.scalar if dvh == 0 else nc.vector
                    ecopy(eng, osh, pf)
                    nc.sync.dma_start(
                        out_d.ap()[
                            ncc * 128 : (ncc + 1) * 128,
                            mi,
                            dvh * 512 : (dvh + 1) * 512,
                        ],
                        osh,
                    )

    nc.compile()
    return nc


_NC_CACHE = None


def _get_nc():
    global _NC_CACHE
    if _NC_CACHE is None:
        _NC_CACHE = build_nc()
    return _NC_CACHE


def _dr_pack(a):
    """[512, ...] rows dq = c*256 + i*128 + p -> [256 = c*128+p, 2 = i, ...]"""
    s = a.shape[1:]
    return a.reshape(2, 2, 128, *s).transpose(0, 2, 1, *range(3, 3 + len(s))).reshape(
        256, 2, *s
    )


def kernel(Q, K, mask, w_q, w_k, w_v, w_o):
    from concourse.bass_utils import run_bass_kernel_spmd

    Q = np.asarray(Q, dtype=np.float32)
    K = np.asarray(K, dtype=np.float32)
    mask = np.asarray(mask)
    w_q = np.asarray(w_q, dtype=np.float32)
    w_k = np.asarray(w_k, dtype=np.float32)
    w_v = np.asarray(w_v, dtype=np.float32)
    w_o = np.asarray(w_o, dtype=np.float32)

    wq8 = np.ascontiguousarray(_dr_pack(w_q * WS)).astype(NPF8)
    wk8 = np.ascontiguousarray(_dr_pack(w_k * WS)).astype(NPF8)
    wvb = w_v.astype(NPBF)
    wob = w_o.astype(NPBF)

    in_maps = []
    for b in range(B):
        qt = np.ascontiguousarray(Q[b].transpose(2, 1, 0))   # [DQ, M, N]
        kt = np.ascontiguousarray(K[b].transpose(2, 1, 0))   # [DK, M, KN]
        in_maps.append(
            {
                "QT8": np.ascontiguousarray(_dr_pack(qt)).astype(NPF8),
                "KT8": np.ascontiguousarray(_dr_pack(kt)).astype(NPF8),
                "KTB": kt.astype(NPBF),
                "MT": np.ascontiguousarray(mask[b].T).astype(NPBF),
                "WQ8": wq8,
                "WK8": wk8,
                "WVB": wvb,
                "WOB": wob,
            }
        )

    nc = _get_nc()
    r = run_bass_kernel_spmd(nc, in_maps, core_ids=list(range(B)), trace=False)
    return np.stack([r.results[b]["out"] for b in range(B)], axis=0)


if __name__ == "__main__":
    rng = np.random.default_rng(0)
    inputs = {
        "Q": rng.standard_normal((B, N, M, DQ), dtype=np.float32),
        "K": rng.standard_normal((B, KN, M, DK), dtype=np.float32),
        "mask": rng.integers(0, 2, (B, N, KN)).astype(np.int32),
        "w_q": (rng.standard_normal((DQ, DV), dtype=np.float32) * 0.02),
        "w_k": (rng.standard_normal((DK, DV), dtype=np.float32) * 0.02),
        "w_v": (rng.standard_normal((DK, DV), dtype=np.float32) * 0.02),
        "w_o": (rng.standard_normal((DV, DV), dtype=np.float32) * 0.02),
    }
    out = kernel(**inputs)
    print("out", out.shape, out.dtype, float(np.abs(out).max()))


# revision 55
# speedup vs baseline: 1.0076x; 1.0076x over previous
"""EquiMHA Trainium2 kernel.

Data-parallel over batch B=8 across the 8 NeuronCores (one batch element per
core, weights replicated, no collectives).

Per-core computation for batch b (N=512, M=4, KN=512, DQ=DK=512, DV=1024,
H=16, D=64):
  Qp = Q[b] @ w_q, Kp = K[b] @ w_k, Vp = K[b] @ w_v
  E[h,n,k] = sum_{m,d} Qp[n,m,h*64+d] Kp[k,m,h*64+d] / 32
  A = masked_softmax(E)        (max-subtraction skipped: |E|/32 <= ~2, and the
                                max cancels exactly up to the +eps term)
  O[n,m,h*64+d] = sum_k A[h,n,k] Vp[k,m,h*64+d]
  out = O @ w_o

Precision strategy (tolerance is 2e-2; measured pipeline error ~8e-3):
  - Q/K-side projections run in fp8e4m3 with DoubleRow perf mode (2x PE
    rate, 256-deep contraction per pass). Host pre-quantizes Q, K and
    w_q, w_k (weights pre-scaled by 64 into fp8 normal range; the 64*64
    factor is folded into the exp scale).
  - Everything downstream (E scores, A@V, O@w_o) runs in bf16, which is
    full PE rate and halves SBUF/DMA vs f32 so the whole working set
    stays on-chip (no DRAM round trips).

Layout strategy: the host pre-transposes all inputs into the exact tile
layouts the PE wants, so the kernel does zero on-chip transposes:
  - QT8/KT8 [256,2,M,N]: [c*128+p, i, m, n] with dq = c*256+i*128+p, the
    DoubleRow pairing of two 128-deep contraction chunks.
  - P1/P2 emit qpp/kpp[h] = [(s,d) 128, mp, n|k] bf16 via partition-shifted
    psum evacuation (DoubleRow outputs land on psum partitions 0-63).
  - E^T[k,n] per head is a 2-matmul bf16 contraction over mp; softmax runs
    in [k, n] orientation entirely OFF the PE: exp on ACT (bf16 out), mask
    multiply + a bf16 partial-sum tree on DVE (2-byte fast mode), a Pool
    partition_all_reduce for the column sums (output already broadcast
    across partitions), and a bf16 DVE reciprocal.
  - Vp2[kc] = [128 k, (h, mp, s, d) 4096] bf16 so the O matmul stationary
    is one contiguous 128-column slice; O psum is normalized on DVE
    directly into OT[hp] = [(h%2,d), m, n] bf16 tiles, which are exactly
    the stationary operands P7 wants. Output leaves in natural [n, m, dvo]
    orientation for contiguous stores.
"""

import numpy as np
import ml_dtypes

import concourse.bacc as bacc
import concourse.mybir as mybir
import concourse.tile as tile

F32 = mybir.dt.float32
F32R = mybir.dt.float32r
F8 = mybir.dt.float8e4
BF = mybir.dt.bfloat16
AF = mybir.ActivationFunctionType
DR = mybir.MatmulPerfMode.DoubleRow

NPF8 = ml_dtypes.float8_e4m3
NPBF = ml_dtypes.bfloat16

B, N, M, KN = 8, 512, 4, 512
DQ, DK, DV, H = 512, 512, 1024, 16
D = DV // H
HP = H // 2          # head pairs (P7 contraction chunks)
KC = KN // 128       # k chunks
NC = N // 128        # n chunks
WS = 64.0            # host pre-scale for fp8 weights
SCALE = 1.0 / 32.0   # 1/sqrt(DV)
ESCALE = SCALE / (WS * WS)  # fused into exp


def build_nc():
    nc = bacc.Bacc("TRN2", target_bir_lowering=False, debug=False, num_devices=8)

    qt8_d = nc.dram_tensor("QT8", [128, 2, 2, M, N], F8, kind="ExternalInput")
    kt8_d = nc.dram_tensor("KT8", [128, 2, 2, M, KN], F8, kind="ExternalInput")
    ktb_d = nc.dram_tensor("KTB", [DK, M, KN], BF, kind="ExternalInput")
    mt_d = nc.dram_tensor("MT", [KN, N], BF, kind="ExternalInput")
    wq8_d = nc.dram_tensor("WQ8", [128, 2, 2, DV], F8, kind="ExternalInput")
    wk8_d = nc.dram_tensor("WK8", [128, 2, 2, DV], F8, kind="ExternalInput")
    wvb_d = nc.dram_tensor("WVB", [DK, DV], BF, kind="ExternalInput")
    wob_d = nc.dram_tensor("WOB", [DV, DV], BF, kind="ExternalInput")
    out_d = nc.dram_tensor("out", [N, M, DV], F32, kind="ExternalOutput")

    def ecopy(eng, dst, src):
        if eng is nc.scalar:
            nc.scalar.copy(dst, src)
        else:
            eng.tensor_copy(dst, src)

    with tile.TileContext(nc) as tc:
        with tc.tile_pool(name="persist", bufs=1) as persist:
            maskT2 = [persist.tile([128, 2, N], BF, name=f"mT{p}") for p in range(2)]
            vp2 = [persist.tile([128, M * DV], BF, name=f"vp2_{kc}") for kc in range(KC)]
            qpp = [persist.tile([128, 2, N], BF, name=f"qpp{h}") for h in range(H)]
            kpp = [persist.tile([128, 2, KN], BF, name=f"kpp{h}") for h in range(H)]
            ot = [persist.tile([128, M, N], BF, name=f"ot{hp}") for hp in range(HP)]
            wob = [persist.tile([128, DV], BF, name=f"wob{hp}") for hp in range(HP)]

            # ============ projections ============
            # Phase A: P2 (fp8 DR, evac-heavy) interleaved with half of P3
            # (bf16, PE-heavy); phase B: P1 interleaved with the other half.
            # Interleaving keeps the psum-evac engines (ACT/DVE) loaded
            # continuously instead of bursting past PE per phase.
            eng_ns = {"act": 0.0, "dve": 0.0}

            def pick_eng(act_cost, dve_cost):
                if eng_ns["act"] + act_cost <= eng_ns["dve"] + dve_cost:
                    eng_ns["act"] += act_cost
                    return nc.scalar
                eng_ns["dve"] += dve_cost
                return nc.vector

            with (
                tc.tile_pool(name="xk8", bufs=1) as xk8p,
                tc.tile_pool(name="w8", bufs=1) as w8p,
                tc.tile_pool(name="xkb", bufs=1) as xkbp,
                tc.tile_pool(name="wvb", bufs=1) as wvbp,
                tc.tile_pool(name="ppj", bufs=3, space="PSUM") as ppj,
                tc.tile_pool(name="ppv", bufs=2, space="PSUM") as ppv,
            ):
                # merged [p, c, ...] tiles: one DMA folds both 128-row
                # blocks of the 256-row dram tensors, minimizing the number
                # of serialized descriptor-generation setups on the SP queue
                xk8 = xk8p.tile([128, 2, 2, M, KN], F8, name="xk8")
                wk8 = w8p.tile([128, 2, 2, DV], F8, name="wk8")
                xkb = [xkbp.tile([128, M, KN], BF, name=f"xkb{c}") for c in range(4)]
                wvb = [wvbp.tile([128, DV], BF, name=f"wvb{c}") for c in range(4)]
                xq8 = xk8p.tile([128, 2, 2, M, N], F8, name="xq8")
                wq8 = w8p.tile([128, 2, 2, DV], F8, name="wq8")
                nc.sync.dma_start(wk8, wk8_d.ap())
                for m in (0, 2, 1, 3):
                    nc.sync.dma_start(
                        xk8[:, :, :, m, :], kt8_d.ap()[:, :, :, m, :]
                    )
                for c in range(4):
                    nc.sync.dma_start(wvb[c], wvb_d.ap()[c * 128 : (c + 1) * 128])
                    nc.sync.dma_start(xkb[c], ktb_d.ap()[c * 128 : (c + 1) * 128])
                nc.sync.dma_start(wq8, wq8_d.ap())
                nc.sync.dma_start(xq8, qt8_d.ap())
                for kc in range(KC):
                    nc.sync.dma_start(
                        maskT2[kc // 2][:, kc % 2, :],
                        mt_d.ap()[kc * 128 : (kc + 1) * 128],
                    )
                for hp in range(HP):
                    nc.sync.dma_start(wob[hp], wob_d.ap()[hp * 128 : (hp + 1) * 128])

                def proj8(h, s, w_sb, x_sb, dst):
                    pq2 = ppj.tile([64, 1024], F32, name="pq2", tag="pq2")
                    for mp in range(2):
                        m = 2 * mp + s
                        for nh in range(2):
                            for c in range(2):
                                nc.tensor.matmul(
                                    pq2[:, mp * 512 + nh * 256 : mp * 512 + (nh + 1) * 256],
                                    w_sb[:, c, :, h * 64 : (h + 1) * 64],
                                    x_sb[:, c, :, m, nh * 256 : (nh + 1) * 256],
                                    start=(c == 0),
                                    stop=(c == 1),
                                    perf_mode=DR,
                                )
                    ecopy(
                        pick_eng(1030, 1310),
                        dst[s * 64 : (s + 1) * 64, :, :],
                        pq2.rearrange("p (t n) -> p t n", t=2),
                    )

                p3_units = [
                    (mi, kc, dvh)
                    for mi in range(M)
                    for kc in range(KC)
                    for dvh in range(2)
                ]

                def p3_unit(u):
                    mi, kc, dvh = u
                    off = (mi // 2) * 128 + (mi % 2) * 64
                    pv = ppv.tile([128, 512], F32, name="pv", tag="pv")
                    for c in range(4):
                        nc.tensor.matmul(
                            pv,
                            xkb[c][:, mi, kc * 128 : (kc + 1) * 128],
                            wvb[c][:, dvh * 512 : (dvh + 1) * 512],
                            start=(c == 0),
                            stop=(c == 3),
                        )
                    v4 = vp2[kc].rearrange("p (h c) -> p h c", h=H)
                    ecopy(
                        pick_eng(610, 730),
                        v4[:, dvh * 8 : (dvh + 1) * 8, off : off + 64],
                        pv.rearrange("p (h d) -> p h d", h=8),
                    )

                p3i = 0
                # phase A: P2, s-major (the s=0 pass needs only half of K);
                # two P3 units per head in the second pass
                for h in range(H):
                    proj8(h, 0, wk8, xk8, kpp[h])
                for h in range(H):
                    proj8(h, 1, wk8, xk8, kpp[h])
                    if h >= 8:
                        p3_unit(p3_units[p3i]); p3i += 1
                        p3_unit(p3_units[p3i]); p3i += 1
                # phase B: P1 x16 heads; one P3 unit per head
                for h in range(H):
                    proj8(h, 0, wq8, xq8, qpp[h])
                    proj8(h, 1, wq8, xq8, qpp[h])
                    if p3i < len(p3_units):
                        p3_unit(p3_units[p3i]); p3i += 1
                while p3i < len(p3_units):
                    p3_unit(p3_units[p3i]); p3i += 1

            # ============ attention, per head (software-pipelined) ============
            # Depth-3 pipeline: iteration h emits norm(h-3), sums(h-1)'s
            # add/all-reduce, E(h), recip(h-1), then O(h-2) — each engine's
            # queue only ever holds instructions whose operands are already
            # (or imminently) ready, so the cross-engine softmax chain
            # (exp -> mask-mul -> adds -> all-reduce -> recip -> normalize)
            # never stalls the PE or head-of-line-blocks DVE. Softmax sums
            # run OFF the PE: bf16 tree-adds on DVE (2-byte fast mode), a
            # Pool partition_all_reduce (output already broadcast across
            # partitions), and a bf16 DVE reciprocal.
            import concourse.bass_isa as bass_isa

            with (
                tc.tile_pool(name="exp", bufs=4) as expp,
                tc.tile_pool(name="emp", bufs=12) as emp,
                tc.tile_pool(name="sump", bufs=2) as sump,
                tc.tile_pool(name="repp", bufs=3) as repp,
                tc.tile_pool(name="opop", bufs=3) as opop,
                tc.tile_pool(name="pse", bufs=2, space="PSUM") as pse,
                tc.tile_pool(name="pso", bufs=2, space="PSUM") as pso,  # 2x2+2x2 = 8
            ):

                def emit_e(h):
                    # E psum lands in 2-bank [128, 1024] pairs so one ACT
                    # exp covers two k-chunks (halves ACT instruction count)
                    em = []
                    for p in range(2):
                        pe2 = pse.tile([128, 2 * N], F32, name="pe", tag="pe")
                        for half in range(2):
                            kc = 2 * p + half
                            for mp in range(2):
                                nc.tensor.matmul(
                                    pe2[:, half * N : (half + 1) * N],
                                    kpp[h][:, mp, kc * 128 : (kc + 1) * 128],
                                    qpp[h][:, mp, :],
                                    start=(mp == 0),
                                    stop=(mp == 1),
                                )
                        ex2 = expp.tile([128, 2, N], BF, name="ex", tag="ex")
                        nc.scalar.activation(
                            ex2.rearrange("p t n -> p (t n)"),
                            pe2,
                            AF.Exp,
                            scale=ESCALE,
                        )
                        for half in range(2):
                            emt = emp.tile([128, N], BF, name="em", tag="em")
                            nc.vector.tensor_mul(
                                emt, ex2[:, half, :], maskT2[p][:, half, :]
                            )
                            em.append(emt)
                    return em

                def emit_sums(h, em):
                    """bf16 tree-adds + Pool all-reduce (recip emitted later)"""
                    t0 = sump.tile([128, N], BF, name="t0", tag="t0")
                    nc.vector.tensor_add(t0, em[0], em[1])
                    t1 = sump.tile([128, N], BF, name="t1", tag="t1")
                    nc.vector.tensor_add(t1, em[2], em[3])
                    s_all = sump.tile([128, N], BF, name="s_all", tag="t0")
                    nc.vector.tensor_add(s_all, t0, t1)
                    s_red = sump.tile([128, N], BF, name="s_red", tag="t1")
                    nc.gpsimd.partition_all_reduce(
                        s_red, s_all, channels=128, reduce_op=bass_isa.ReduceOp.add
                    )
                    return s_red

                def emit_recip(s_red):
                    rep = repp.tile([128, N], BF, name="rep", tag="rep")
                    with nc.allow_low_precision(reason="softmax 1/sum"):
                        nc.vector.reciprocal(rep, s_red)
                    return rep

                def emit_o(h, em):
                    """O matmuls into a 2-bank psum pair; one ACT evac"""
                    po2 = pso.tile([128, 2 * N], F32, name="po", tag="po")
                    for mp in range(2):
                        for kc in range(KC):
                            nc.tensor.matmul(
                                po2[:, mp * N : (mp + 1) * N],
                                vp2[kc][:, h * 256 + mp * 128 : h * 256 + (mp + 1) * 128],
                                em[kc],
                                start=(kc == 0),
                                stop=(kc == KC - 1),
                            )
                    opo = opop.tile([128, 2, N], BF, name="opo", tag="opo")
                    nc.scalar.copy(
                        opo.rearrange("p t n -> p (t n)"), po2
                    )
                    return opo

                def emit_norm(h, opo, rep):
                    """normalize O into OT tiles (bf16 fast muls, DVE/Pool;
                    late heads all-DVE so the P7 warm-up isn't gated on Pool)"""
                    hp, hs = h // 2, h % 2
                    for mp in range(2):
                        for s in range(2):
                            eng = nc.gpsimd if (mp == 1 and h < 13) else nc.vector
                            eng.tensor_mul(
                                ot[hp][hs * 64 : (hs + 1) * 64, 2 * mp + s, :],
                                opo[s * 64 : (s + 1) * 64, mp, :],
                                rep[s * 64 : (s + 1) * 64, :],
                            )

                em_q, sr_q, rep_q, opo_q = {}, {}, {}, {}
                for h in range(H):
                    if h >= 3:
                        emit_norm(h - 3, opo_q.pop(h - 3), rep_q[h - 3])
                    if h >= 1:
                        sr_q[h - 1] = emit_sums(h - 1, em_q[h - 1])
                    if h >= 14:
                        # last iterations: O before E so the opo evacs reach
                        # ACT ahead of the exps and free the psum pairs the
                        # epilogue's O(14)/O(15) need
                        opo_q[h - 2] = emit_o(h - 2, em_q.pop(h - 2))
                    em_q[h] = emit_e(h)
                    if h >= 1:
                        rep_q[h - 1] = emit_recip(sr_q.pop(h - 1))
                    if 2 <= h < 14:
                        opo_q[h - 2] = emit_o(h - 2, em_q.pop(h - 2))
                # epilogue: h-1=15 sums/recip, O(14), O(15), norms 13..15
                sr_q[15] = emit_sums(15, em_q[15])
                rep_q[15] = emit_recip(sr_q.pop(15))
                opo_q[14] = emit_o(14, em_q.pop(14))
                emit_norm(13, opo_q.pop(13), rep_q[13])
                opo_q[15] = emit_o(15, em_q.pop(15))
                emit_norm(14, opo_q.pop(14), rep_q[14])
                emit_norm(15, opo_q.pop(15), rep_q[15])

            # ============ P7: output projection (bf16) ============
            # The first four psum groups run contraction chunks hp=0..5
            # before any hp>=6 step, so the PE keeps streaming while the
            # last two heads\' normalized OT tiles are still being written.
            with (
                tc.tile_pool(name="outst", bufs=4) as outstp,
                tc.tile_pool(name="psf", bufs=4, space="PSUM") as psf,
            ):
                units = [(mi, ncc) for mi in range(M) for ncc in range(NC)]

                def p7_mms(pf, mi, ncc, dvh, hps, first, last):
                    for hp in hps:
                        nc.tensor.matmul(
                            pf,
                            ot[hp][:, mi, ncc * 128 : (ncc + 1) * 128],
                            wob[hp][:, dvh * 512 : (dvh + 1) * 512],
                            start=(hp == first),
                            stop=(hp == last),
                        )

                def p7_finish(u, pf0, pf1):
                    mi, ncc = u
                    ost = outstp.tile([128, 2, 512], F32, name="ost", tag="ost")
                    nc.scalar.copy(ost[:, 0, :], pf0)
                    nc.vector.tensor_copy(ost[:, 1, :], pf1)
                    nc.sync.dma_start(
                        out_d.ap()[ncc * 128 : (ncc + 1) * 128, mi, :],
                        ost.rearrange("p a b -> p (a b)"),
                    )

                # warm stretch: 4 psum groups of hp0..5 for the first 2 units
                warm = []
                for u in units[:4]:
                    mi, ncc = u
                    pfs = []
                    for dvh in range(2):
                        pf = psf.tile([128, 512], F32, name="pf", tag="pf")
                        p7_mms(pf, mi, ncc, dvh, range(6), 0, HP - 1)
                        pfs.append(pf)
                    warm.append((u, pfs))
                for u, pfs in warm:
                    mi, ncc = u
                    for dvh in range(2):
                        p7_mms(pfs[dvh], mi, ncc, dvh, range(6, HP), 0, HP - 1)
                    p7_finish(u, *pfs)
                for u in units[4:-1]:
                    mi, ncc = u
                    pfs = []
                    for dvh in range(2):
                        pf = psf.tile([128, 512], F32, name="pf", tag="pf")
                        p7_mms(pf, mi, ncc, dvh, range(HP), 0, HP - 1)
                        pfs.append(pf)
                    p7_finish(u, *pfs)
                # last unit: quarter-column evac/DMA pipeline for a
                # short end-of-kernel drain
                mi, ncc = units[-1]
                for dvh in range(2):
                    pf = psf.tile([128, 512], F32, name="pf", tag="pf")
                    p7_mms(pf, mi, ncc, dvh, range(HP), 0, HP - 1)
                    osh = outstp.tile([128, 512], F32, name="osh", tag="osh")
                    eng = nc.scalar if dvh == 0 else nc.vector
                    ecopy(eng, osh, pf)
                    nc.sync.dma_start(
                        out_d.ap()[
                            ncc * 128 : (ncc + 1) * 128,
                            mi,
                            dvh * 512 : (dvh + 1) * 512,
                        ],
                        osh,
                    )

    nc.compile()
    return nc


_NC_CACHE = None


def _get_nc():
    global _NC_CACHE
    if _NC_CACHE is None:
        _NC_CACHE = build_nc()
    return _NC_CACHE


def _dr_pack(a):
    """[512, ...] rows dq = c*256 + i*128 + p -> [128 = p, 2 = c, 2 = i, ...]"""
    s = a.shape[1:]
    return a.reshape(2, 2, 128, *s).transpose(2, 0, 1, *range(3, 3 + len(s)))


def kernel(Q, K, mask, w_q, w_k, w_v, w_o):
    from concourse.bass_utils import run_bass_kernel_spmd

    Q = np.asarray(Q, dtype=np.float32)
    K = np.asarray(K, dtype=np.float32)
    mask = np.asarray(mask)
    w_q = np.asarray(w_q, dtype=np.float32)
    w_k = np.asarray(w_k, dtype=np.float32)
    w_v = np.asarray(w_v, dtype=np.float32)
    w_o = np.asarray(w_o, dtype=np.float32)

    wq8 = np.ascontiguousarray(_dr_pack(w_q * WS)).astype(NPF8)
    wk8 = np.ascontiguousarray(_dr_pack(w_k * WS)).astype(NPF8)
    wvb = w_v.astype(NPBF)
    wob = w_o.astype(NPBF)

    in_maps = []
    for b in range(B):
        qt = np.ascontiguousarray(Q[b].transpose(2, 1, 0))   # [DQ, M, N]
        kt = np.ascontiguousarray(K[b].transpose(2, 1, 0))   # [DK, M, KN]
        in_maps.append(
            {
                "QT8": np.ascontiguousarray(_dr_pack(qt)).astype(NPF8),
                "KT8": np.ascontiguousarray(_dr_pack(kt)).astype(NPF8),
                "KTB": kt.astype(NPBF),
                "MT": np.ascontiguousarray(mask[b].T).astype(NPBF),
                "WQ8": wq8,
                "WK8": wk8,
                "WVB": wvb,
                "WOB": wob,
            }
        )

    nc = _get_nc()
    r = run_bass_kernel_spmd(nc, in_maps, core_ids=list(range(B)), trace=False)
    return np.stack([r.results[b]["out"] for b in range(B)], axis=0)


if __name__ == "__main__":
    rng = np.random.default_rng(0)
    inputs = {
        "Q": rng.standard_normal((B, N, M, DQ), dtype=np.float32),
        "K": rng.standard_normal((B, KN, M, DK), dtype=np.float32),
        "mask": rng.integers(0, 2, (B, N, KN)).astype(np.int32),
        "w_q": (rng.standard_normal((DQ, DV), dtype=np.float32) * 0.02),
        "w_k": (rng.standard_normal((DK, DV), dtype=np.float32) * 0.02),
        "w_v": (rng.standard_normal((DK, DV), dtype=np.float32) * 0.02),
        "w_o": (rng.standard_normal((DV, DV), dtype=np.float32) * 0.02),
    }
    out = kernel(**inputs)
    print("out", out.shape, out.dtype, float(np.abs(out).max()))
